# revision 1
# baseline (speedup 1.0000x reference)
"""Trainium2 Bass kernel for nn_ASVT_9500467658791 (ragged segment attention).

Pipeline (per point-cloud segment, one segment per NeuronCore, 8 cores):
  q/k/v = feat @ {Wq,Wk,Wv}  (1x1 convs)
  per-segment unscaled-softmax attention  r = softmax(q k^T) v
  t = r @ Wt ; BatchNorm over the full batch (training stats, synced across
  cores via tiny AllGathers) ; out = feat + relu(bn(t))

v3 design notes (fp16 matmul path, PE-lean, stall-free):
  * Everything d-major on chip; the host pre-transposes feat (fp16) and the
    output returns d-major fp16 (host casts/transposes back).
  * The matmul path runs fp16 (same PE rate as bf16 -- 1 col/cycle -- but
    10-bit mantissa keeps the softmax scores accurate; fp32r would run the
    PE in 2-pass HIGH mode at half the column rate).  exp outputs stay bf16
    for range (unnormalized exp reaches e^25).
  * Scores are computed keys-major in groups of 4 key tiles issued
    back-to-back with tile_position row packing (32-contraction), so the 4
    matmuls overlap in the PE array (~3x).
  * exp() runs on ACT over PAIRS of score banks (one [128,1024] read per
    two key tiles) to keep ACT under the PE rate.
  * Softmax denominators never touch the PE per key tile: the exp'd tiles
    are summed elementwise (pair adds split DVE/GpSimd + an f32 tree on
    DVE) and a single ones-stationary matmul per query chunk performs the
    final cross-partition reduction (its all-rows-identical output doubles
    as the partition broadcast for the reciprocal).  Padded keys contribute
    exp(0)=1 each; the host passes -(LP-L)/128 per partition and the tree's
    last add subtracts it, so no key-direction masking exists anywhere.
  * The per-chunk epilogue (tree tail / denominator / reciprocal / rT
    scale / tT matmuls / BN partial sums) is emitted AFTER the next chunk's
    first two score groups, so the PE never drains at chunk boundaries
    (which would also re-throttle the HAM clock gate).
  * BN statistics live in one [128, 4*nch] accumulator tile, are packed
    [128,4] with a single strided reduce (no PE transpose) and AllGathered
    in two partials; the combine is a strided reduce over gathered rows and
    the scale/bias math is vectorized over both d-halves ([128,2] tiles).
  * BN apply is split across engines: half the tiles via ACT Relu + DVE
    residual add, half via two DVE scalar_tensor_tensor ops using
    out = max(t*s + (b + feat), feat)  ==  feat + relu(t*s + b).
  * Input DMAs are spread across four engine queues so the ~20 descriptors
    don't serialize on one queue ahead of the first projections.
"""

import os
from collections import deque
from contextlib import ExitStack

import numpy as np

import concourse.bass as bass
import concourse.bacc as bacc
import concourse.tile as tile
from concourse import mybir
from concourse import bass_utils

f32 = mybir.dt.float32
bf16 = mybir.dt.bfloat16
fp16 = mybir.dt.float16
AF = mybir.ActivationFunctionType
ALU = mybir.AluOpType
AX = mybir.AxisListType

NCORES = 8
D = 256
N_TOT = 16384
EPS = 1e-5
LP_MIN = 2176     # default segment pad (17 tiles); raised dynamically if needed

LAST_RESULT = None  # BassKernelResults of the most recent run (for test harness)
_NC_CACHE = {}


def _chunks(LP):
    out, c0 = [], 0
    while c0 < LP:
        out.append((c0, min(512, LP - c0)))
        c0 += 512
    return out


def build_nc(LP):
    NT = LP // 128
    chunks = _chunks(LP)
    nch = len(chunks)
    groups = [list(range(g, min(g + 4, NT))) for g in range(0, NT, 4)]

    nc = bacc.Bacc("TRN2", target_bir_lowering=False, debug=False,
                   enable_asserts=True, num_devices=NCORES)

    featB_d = nc.dram_tensor("featB", [D, LP], fp16, kind="ExternalInput")
    maskr_d = nc.dram_tensor("maskr", [1, LP], bf16, kind="ExternalInput")
    wqk_d = nc.dram_tensor("wqk", [D, 256], fp16, kind="ExternalInput")
    wvt_d = nc.dram_tensor("wvt", [D, 512], fp16, kind="ExternalInput")
    cst_d = nc.dram_tensor("cst", [128, 5], f32, kind="ExternalInput")
    out_d = nc.dram_tensor("out", [D, LP], fp16, kind="ExternalOutput")

    # two partial-stats collectives + one warm-up
    cc_in = [nc.dram_tensor(f"cc_in{j}", [128, 4], f32, kind="Internal")
             for j in range(2)]
    cc_out = [nc.dram_tensor(f"cc_out{j}", [128 * NCORES, 4], f32,
                             kind="Internal", addr_space="Shared")
              for j in range(2)]
    ccw_in = nc.dram_tensor("ccw_in", [1, 128], f32, kind="Internal")
    ccw_out = nc.dram_tensor("ccw_out", [NCORES, 128], f32, kind="Internal",
                             addr_space="Shared")

    with tile.TileContext(nc) as tc, ExitStack() as ctx:
        const = ctx.enter_context(tc.tile_pool(name="const", bufs=1))
        big = ctx.enter_context(tc.tile_pool(name="big", bufs=1))
        vpool = ctx.enter_context(tc.tile_pool(name="vpool", bufs=1))
        epool = ctx.enter_context(tc.tile_pool(name="epool", bufs=6))
        padd = ctx.enter_context(tc.tile_pool(name="padd", bufs=1))
        work = ctx.enter_context(tc.tile_pool(name="work", bufs=2))
        small = ctx.enter_context(tc.tile_pool(name="small", bufs=2))
        fpool = ctx.enter_context(tc.tile_pool(name="fpool", bufs=3))
        # PSUM: 8 banks = 2 score-pair tiles (2 banks each) + 2x2 rT accum
        psP = ctx.enter_context(tc.tile_pool(name="psP", bufs=2, space="PSUM"))
        psV = ctx.enter_context(tc.tile_pool(name="psV", bufs=2, space="PSUM"))

        # ---------- input DMAs first, spread over 4 queues ----------
        wq4_sb = [const.tile([128, 128], fp16, tag=f"wq4{h}", name=f"wq4{h}")
                  for h in range(2)]
        wk4_sb = [const.tile([128, 128], fp16, tag=f"wk4{h}", name=f"wk4{h}")
                  for h in range(2)]
        for h in range(2):
            sl = slice(128 * h, 128 * (h + 1))
            nc.sync.dma_start(out=wq4_sb[h], in_=wqk_d[sl, 0:128])
            nc.sync.dma_start(out=wk4_sb[h], in_=wqk_d[sl, 128:256])
        featB_sb = [big.tile([128, LP], fp16, tag=f"featB{h}",
                             name=f"featB{h}") for h in range(2)]
        nc.scalar.dma_start(out=featB_sb[0], in_=featB_d[0:128, :])
        nc.gpsimd.dma_start(out=featB_sb[1], in_=featB_d[128:256, :])

        wv_sb = [const.tile([128, D], fp16, tag=f"wv{h}", name=f"wv{h}")
                 for h in range(2)]
        wt_sb = [const.tile([128, D], fp16, tag=f"wt{h}", name=f"wt{h}")
                 for h in range(2)]
        for h in range(2):
            sl = slice(128 * h, 128 * (h + 1))
            nc.sync.dma_start(out=wv_sb[h], in_=wvt_d[sl, 0:256])
            nc.sync.dma_start(out=wt_sb[h], in_=wvt_d[sl, 256:512])
        # cst columns: gam2 (0:2), bet2 (2:4), ninvn (4:5)
        cst_sb = const.tile([128, 5], f32, tag="cst")
        nc.sync.dma_start(out=cst_sb, in_=cst_d[:, :])
        gam2, bet2 = cst_sb[:, 0:2], cst_sb[:, 2:4]
        ninvn_sb = cst_sb[:, 4:5]
        maskbc_sb = const.tile([128, LP], bf16, tag="maskbc")
        _mr = maskr_d[0:1, :]
        nc.sync.dma_start(out=maskbc_sb, in_=bass.AP(
            tensor=_mr.tensor, offset=_mr.offset, ap=[[0, 128]] + list(_mr.ap[1:])))

        # PE clock warm-up on a memset constant (no DMA dependency; covers the
        # ~3.4us HAM window while the input DMAs stream in)
        warm_c = const.tile([128, 128], bf16, tag="warm_c")
        nc.vector.memset(warm_c, 0.5)
        ps_w = psP.tile([128, 1024], f32, tag="sp", name="ps_w")
        for i in range(14):
            nc.tensor.matmul(ps_w[:, 0:128], lhsT=warm_c, rhs=warm_c,
                             start=True, stop=True)
        warm_junk = const.tile([128, 1], f32, tag="warm_junk")
        nc.vector.tensor_copy(out=warm_junk, in_=ps_w[:, 0:1])

        ones_b = const.tile([128, 128], bf16, tag="ones_b")
        nc.vector.memset(ones_b, 1.0)

        # ---------- warm-up collective (runs on TOPSP during phase A) -------
        wz = const.tile([1, 128], f32, tag="wz")
        nc.vector.memset(wz, 0.0)
        nc.sync.dma_start(out=ccw_in[:, :], in_=wz)
        nc.gpsimd.collective_compute(
            "AllGather", ALU.bypass, replica_groups=[list(range(NCORES))],
            ins=[ccw_in[:, :]], outs=[ccw_out[:, :]])

        # ---------- phase A: projections (all fp16) ----------
        qT_sb = big.tile([128, LP], fp16, tag="qT", name="qT")
        kT_sb = big.tile([128, LP], fp16, tag="kT", name="kT")
        v_sb = []
        for ci, (c0, cw) in enumerate(chunks):
            csl = slice(c0, c0 + cw)
            for wrep, dst in ((wq4_sb, qT_sb), (wk4_sb, kT_sb)):
                ps = psP.tile([128, 1024], f32, tag="sp", name=f"psqk{ci}")
                nc.tensor.matmul(ps[:, :cw], lhsT=wrep[0],
                                 rhs=featB_sb[0][:, csl],
                                 start=True, stop=False)
                nc.tensor.matmul(ps[:, :cw], lhsT=wrep[1],
                                 rhs=featB_sb[1][:, csl],
                                 start=False, stop=True)
                nc.vector.tensor_copy(out=dst[:, csl], in_=ps[:, :cw])
            # v for the key tiles inside this chunk
            for i in range(c0 // 128, (c0 + cw) // 128):
                ksl = slice(128 * i, 128 * (i + 1))
                ps = psP.tile([128, 1024], f32, tag="sp", name=f"psv{i}")
                nc.tensor.matmul(ps[:, 0:D], lhsT=featB_sb[0][:, ksl],
                                 rhs=wv_sb[0], start=True, stop=False)
                nc.tensor.matmul(ps[:, 0:D], lhsT=featB_sb[1][:, ksl],
                                 rhs=wv_sb[1], start=False, stop=True)
                vt = vpool.tile([128, D], fp16, tag=f"v{i}", name=f"v{i}")
                nc.vector.tensor_copy(out=vt, in_=ps[:, 0:D])
                v_sb.append(vt)

        rT_sb = [big.tile([128, LP], fp16, tag=f"rT{h}", name=f"rT{h}")
                 for h in range(2)]
        tT_sb = [big.tile([128, LP], fp16, tag=f"tT{h}", name=f"tT{h}")
                 for h in range(2)]
        # BN partial sums: col = (2s + h) * nch + ci  for s in {sum, sumsq}
        sums_all = const.tile([128, 4 * nch], f32, tag="sums_all")

        def emit_stats_ag(j, lo, hi):
            """Pack partial sums over chunks [lo, hi) and AllGather them."""
            stf = const.tile([128, 4], f32, tag=f"stf{j}", name=f"stf{j}")
            _s = sums_all[:, lo:lo + 1]
            nc.vector.reduce_sum(
                out=stf,
                in_=bass.AP(tensor=_s.tensor, offset=_s.offset,
                            ap=[list(_s.ap[0]), [nch, 4], [1, hi - lo]]),
                axis=AX.X)
            nc.sync.dma_start(out=cc_in[j][:, :], in_=stf)
            nc.gpsimd.collective_compute(
                "AllGather", ALU.bypass, replica_groups=[list(range(NCORES))],
                ins=[cc_in[j][:, :]], outs=[cc_out[j][:, :]])

        # ---------- phases B-D: attention + r^T + t^T, chunked over queries,
        # with the per-chunk epilogue deferred past the next chunk's start ---
        pend = deque()     # (ci, kts, ps_rt) groups whose rT mms aren't issued
        tails = deque()    # deferred per-chunk epilogues

        def issue_rt(ci, kts, ps_rt, etof):
            cw = chunks[ci][1]
            for kt in kts:
                et, off = etof[kt]
                for h in range(2):
                    nc.tensor.matmul(ps_rt[h][:, :cw],
                                     lhsT=v_sb[kt][:, 128 * h:128 * (h + 1)],
                                     rhs=et[:, off:off + cw],
                                     start=(kt == 0), stop=(kt == NT - 1))

        for ci, (c0, cw) in enumerate(chunks):
            csl = slice(c0, c0 + cw)
            ps_rt = [psV.tile([128, 512], f32, tag=f"v{h}",
                              name=f"psrt{ci}_{h}") for h in range(2)]
            etof = {}
            psums = []
            single = [None]
            addc = 0

            for g, kts in enumerate(groups):
                npair = (len(kts) + 1) // 2
                pts = [psP.tile([128, 1024], f32, tag="sp",
                                name=f"pt{ci}_{g}_{pi}") for pi in range(npair)]
                for j, kt in enumerate(kts):
                    b = kt % 4
                    nc.tensor.matmul(
                        pts[j // 2][:, (j % 2) * 512:(j % 2) * 512 + cw],
                        lhsT=kT_sb[32 * b:32 * b + 32, 128 * kt:128 * (kt + 1)],
                        rhs=qT_sb[32 * b:32 * b + 32, c0:c0 + cw],
                        start=True, stop=True, tile_position=(32 * b, 0))
                for pi in range(npair):
                    pkts = kts[2 * pi:2 * pi + 2]
                    et = epool.tile([128, 1024], bf16, tag="e",
                                    name=f"e{ci}_{g}_{pi}")
                    if len(pkts) == 2:
                        if cw == 512:
                            nc.scalar.activation(out=et[:, 0:1024],
                                                 in_=pts[pi][:, 0:1024],
                                                 func=AF.Exp)
                        else:
                            _p = pts[pi]
                            in_ap = bass.AP(
                                tensor=_p.tensor, offset=_p.offset,
                                ap=[list(_p.ap[0]), [512, 2], [1, cw]])
                            _e = et
                            out_ap = bass.AP(
                                tensor=_e.tensor, offset=_e.offset,
                                ap=[list(_e.ap[0]), [cw, 2], [1, cw]])
                            nc.scalar.activation(out=out_ap, in_=in_ap,
                                                 func=AF.Exp)
                        etof[pkts[0]] = (et, 0)
                        etof[pkts[1]] = (et, cw)
                        # pair add: split between GpSimd and DVE
                        ps8 = padd.tile([128, 512], f32, tag=f"p{g}_{pi}",
                                        name=f"p{ci}_{g}_{pi}")
                        eng = nc.gpsimd if (addc % 2 == 0) else nc.vector
                        addc += 1
                        eng.tensor_add(out=ps8[:, :cw], in0=et[:, 0:cw],
                                       in1=et[:, cw:cw + cw])
                        psums.append(ps8)
                    else:
                        nc.scalar.activation(out=et[:, 0:cw],
                                             in_=pts[pi][:, 0:cw], func=AF.Exp)
                        etof[pkts[0]] = (et, 0)
                        single[0] = et
                pend.append((ci, kts, ps_rt, etof))
                if len(pend) > 2:
                    issue_rt(*pend.popleft())
                if g == 1 and tails:
                    tails.popleft()()

            def make_tail(ci=ci, c0=c0, cw=cw, csl=csl, ps_rt=ps_rt,
                          psums=psums, single=single):
                def tail():
                    # denominator tree tail (DVE) + padded-key fixup
                    cur, lvl = psums, 0
                    while len(cur) > 2 or (len(cur) == 2
                                           and single[0] is not None):
                        nxt = []
                        for j in range(0, len(cur) - 1, 2):
                            t = padd.tile([128, 512], f32, tag=f"t{lvl}_{j}",
                                          name=f"t{ci}_{lvl}_{j}")
                            nc.vector.tensor_add(out=t[:, :cw],
                                                 in0=cur[j][:, :cw],
                                                 in1=cur[j + 1][:, :cw])
                            nxt.append(t)
                        if len(cur) % 2:
                            nxt.append(cur[-1])
                        cur, lvl = nxt, lvl + 1
                    esum = work.tile([128, 512], bf16, tag="esum")
                    fin1 = single[0] if single[0] is not None else cur[1]
                    nc.vector.scalar_tensor_tensor(
                        out=esum[:, :cw], in0=cur[0][:, :cw], scalar=ninvn_sb,
                        in1=fin1[:, :cw], op0=ALU.add, op1=ALU.add)

                    # cross-partition reduction + broadcast via ones-matmul
                    ps_d = psP.tile([128, 1024], f32, tag="sp",
                                    name=f"psd{ci}")
                    nc.tensor.matmul(ps_d[:, :cw], lhsT=ones_b,
                                     rhs=esum[:, :cw], start=True, stop=True)

                    rec = work.tile([128, 512], f32, tag="recd")
                    nc.vector.reciprocal_approx_fast(out=rec[:, :cw],
                                                     in_=ps_d[:, :cw])
                    nc.vector.tensor_mul(out=rec[:, :cw], in0=rec[:, :cw],
                                         in1=maskbc_sb[:, csl])
                    for h in range(2):
                        nc.vector.tensor_mul(out=rT_sb[h][:, csl],
                                             in0=ps_rt[h][:, :cw],
                                             in1=rec[:, :cw])

                    # t^T = Wt^T @ rT + BN partial stats
                    ps_t = psP.tile([128, 1024], f32, tag="sp",
                                    name=f"pst{ci}")
                    for h in range(2):
                        hsl = slice(128 * h, 128 * (h + 1))
                        off = 512 * h
                        nc.tensor.matmul(ps_t[:, off:off + cw],
                                         lhsT=wt_sb[0][:, hsl],
                                         rhs=rT_sb[0][:, csl],
                                         start=True, stop=False)
                        nc.tensor.matmul(ps_t[:, off:off + cw],
                                         lhsT=wt_sb[1][:, hsl],
                                         rhs=rT_sb[1][:, csl],
                                         start=False, stop=True)
                    for h in range(2):
                        off = 512 * h
                        nc.vector.tensor_scalar(
                            out=tT_sb[h][:, csl], in0=ps_t[:, off:off + cw],
                            scalar1=1.0, scalar2=0.0, op0=ALU.mult,
                            op1=ALU.add,
                            accum_out=sums_all[:, h * nch + ci:
                                               h * nch + ci + 1])
                        sqj = work.tile([128, 512], fp16, tag=f"sqj{h}",
                                        name=f"sqj{h}_{ci}")
                        nc.vector.scalar_tensor_tensor(
                            out=sqj[:, :cw], in0=tT_sb[h][:, csl], scalar=0.0,
                            in1=tT_sb[h][:, csl], op0=ALU.add, op1=ALU.mult,
                            accum_out=sums_all[:, (2 + h) * nch + ci:
                                               (2 + h) * nch + ci + 1])
                    # partial-stats collective: overlaps remaining compute
                    if ci == nch - 3:
                        emit_stats_ag(0, 0, nch - 2)
                return tail
            tails.append(make_tail())

        while pend:
            issue_rt(*pend.popleft())
        while tails:
            tails.popleft()()

        # ---------- phase E: last partial + combine global BN stats ----------
        emit_stats_ag(1, nch - 2, nch)
        ag_all = const.tile([128, 64], f32, tag="ag_all")
        for j in range(2):
            _t = cc_out[j][:, :]
            # [8c*128p, 4s] -> [128p, 8c, 4s]
            nc.sync.dma_start(
                out=ag_all[:, 32 * j:32 * (j + 1)],
                in_=bass.AP(tensor=_t.tensor, offset=_t.offset,
                            ap=[[4, 128], [512, 8], [1, 4]]))
        stats4 = const.tile([128, 4], f32, tag="stats4")
        for s in range(4):
            base = ag_all[:, s:s + 1]
            nc.vector.reduce_sum(
                out=stats4[:, s:s + 1],
                in_=bass.AP(tensor=base.tensor, offset=base.offset,
                            ap=[list(base.ap[0]), [4, 16]]),
                axis=AX.X)

        # scale/bias vectorized over both halves: [128, 2], short dep chain
        inv_n = 1.0 / float(N_TOT)
        s4n = small.tile([128, 4], f32, tag="s4n")
        nc.vector.tensor_scalar_mul(out=s4n, in0=stats4, scalar1=inv_n)
        mu = s4n[:, 0:2]
        musq = small.tile([128, 2], f32, tag="musq")
        nc.vector.tensor_mul(out=musq, in0=mu, in1=mu)
        varp = small.tile([128, 2], f32, tag="varp")
        # varp = (E[t^2] + EPS) - mu^2
        nc.vector.scalar_tensor_tensor(out=varp, in0=s4n[:, 2:4], scalar=EPS,
                                       in1=musq, op0=ALU.add,
                                       op1=ALU.subtract)
        sd = small.tile([128, 2], f32, tag="sd")
        nc.scalar.activation(out=sd, in_=varp, func=AF.Sqrt)
        rsig = small.tile([128, 2], f32, tag="rsig")
        nc.vector.reciprocal(out=rsig, in_=sd)
        sc2 = small.tile([128, 2], f32, tag="sc2")
        nc.vector.tensor_mul(out=sc2, in0=rsig, in1=gam2)
        bi2 = small.tile([128, 2], f32, tag="bi2")
        nc.vector.tensor_mul(out=bi2, in0=mu, in1=sc2)
        nc.vector.tensor_sub(out=bi2, in0=bet2, in1=bi2)

        # ---------- phase F: BN apply + relu + residual (fp16, d-major) -----
        fslices = [(c0, min(1024, LP - c0)) for c0 in range(0, LP, 1024)]
        for h in range(2):
            sc_h, bi_h = sc2[:, h:h + 1], bi2[:, h:h + 1]
            for fi, (f0, fw) in enumerate(fslices):
                qsl = slice(f0, f0 + fw)
                relu_t = fpool.tile([128, 1024], fp16, tag=f"r{h}",
                                    name=f"relu{h}_{fi}")
                nc.scalar.activation(out=relu_t[:, :fw], in_=tT_sb[h][:, qsl],
                                     func=AF.Relu, bias=bi_h, scale=sc_h)
                o = fpool.tile([128, 1024], fp16, tag=f"o{h}",
                               name=f"o{h}_{fi}")
                nc.vector.tensor_add(out=o[:, :fw], in0=relu_t[:, :fw],
                                     in1=featB_sb[h][:, qsl])
                nc.sync.dma_start(out=out_d[128 * h:128 * (h + 1), qsl],
                                  in_=o[:, :fw])

    nc.compile()
    return nc


def _get_nc(LP):
    if LP not in _NC_CACHE:
        _NC_CACHE[LP] = build_nc(LP)
    return _NC_CACHE[LP]


def kernel(**inputs):
    global LAST_RESULT
    feat = np.asarray(inputs["feat"], dtype=np.float32)
    bids = np.asarray(inputs["bids"])
    Wq = np.asarray(inputs["Wq"], dtype=np.float32)
    Wk = np.asarray(inputs["Wk"], dtype=np.float32)
    Wv = np.asarray(inputs["Wv"], dtype=np.float32)
    Wt = np.asarray(inputs["Wt"], dtype=np.float32)
    gamma = np.asarray(inputs["gamma"], dtype=np.float32)
    beta = np.asarray(inputs["beta"], dtype=np.float32)

    n, d = feat.shape
    assert d == D
    starts = np.searchsorted(bids, np.arange(NCORES)).astype(np.int64)
    ends = np.append(starts[1:], n)
    lens = (ends - starts).astype(np.int64)
    maxlen = int(lens.max())
    LP = max(LP_MIN, ((maxlen + 127) // 128) * 128)
    nc = _get_nc(LP)

    wqk = np.concatenate([Wq] * 4 + [Wk] * 4, axis=1).astype(np.float16)
    wvt = np.concatenate([Wv, Wt], axis=1).astype(np.float16)

    in_maps = []
    for c in range(NCORES):
        seg = feat[starts[c]:ends[c]]
        L = seg.shape[0]
        featB = np.zeros((D, LP), dtype=np.float16)
        featB[:, :L] = seg.T.astype(np.float16)
        import ml_dtypes
        maskr = np.zeros((1, LP), dtype=ml_dtypes.bfloat16)
        maskr[0, :L] = 1.0
        cst = np.empty((128, 5), dtype=np.float32)
        cst[:, 0:2] = gamma.reshape(2, 128).T
        cst[:, 2:4] = beta.reshape(2, 128).T
        cst[:, 4] = -float(LP - L) / 128.0
        in_maps.append({
            "featB": featB, "maskr": maskr, "wqk": wqk, "wvt": wvt,
            "cst": cst,
        })

    trace_cores = None
    if os.environ.get("BASS_TRACE"):
        trace_cores = list(range(NCORES))
    res = None
    for attempt in range(3):
        try:
            res = bass_utils.run_bass_kernel_spmd(
                nc, in_maps, core_ids=list(range(NCORES)),
                trace_cores=trace_cores)
            break
        except Exception:
            # transient device wedge (e.g. NRT_EXEC_UNIT_UNRECOVERABLE from a
            # previous session) — re-initialize the PJRT client and retry
            if attempt == 2:
                raise
            try:
                import jax
                jax.clear_caches()
                jax._src.xla_bridge.backends.cache_clear()  # type: ignore
            except Exception:
                pass
    LAST_RESULT = res

    out = np.empty((n, D), dtype=np.float32)
    for c in range(NCORES):
        o = np.asarray(res.results[c]["out"]).astype(np.float32)
        out[starts[c]:ends[c]] = o.T[:lens[c]]
    return out

